# revision 4
# baseline (speedup 1.0000x reference)
"""Trainium2 Bass kernel for ExpertMLP: out = relu(x @ W_fc.T)^2 @ W_proj.T.

Sharding: 4-way tokens x 2-way hidden across 8 NeuronCores; host sums the
two fp16 hidden-half partials per token while unsharding.

Per-core kernel (T_S=2048 tokens, HID_S=2048 hidden, DIM=1024), fp16
operands, fp32 PSUM accumulation:
  mm1: h^T[j, t] = wfc-chunk.T @ x-chunks     (PSUM accum over d)
  act: relu^2 (ScalarE relu PSUM->SBUF fp16, VectorE square)
  mm2: out^T[d, t] = wproj-chunk.T @ h^T-chunks (PSUM accum over j)

Key optimizations vs naive:
  - Stationary sharing with REAL LDWEIGHTS dedup: bass legalization inserts
    an InstLdweights before every matmul; dedup_ldweights() deletes the
    redundant ones post-scheduling so each 128x128 weight chunk is loaded
    once per 4 streaming matmuls (hardware retains PE-array weights).
  - DRAM layouts pre-swizzled on host so every DMA is large and contiguous
    per partition; DMA issue order = need order (wfc[0:2] first, then x
    k-major with k=0 split per t-chunk) and mm1's first two j-groups are
    interleaved k-wise, so real matmuls start ~0.4us in and the PE never
    starves on the x prologue (no dummy warmup matmuls needed).
  - fp16 partial outputs (halves output DMA; host sums in fp32).
  - PSUM pool parity flipped between phases to avoid a WAW stall at the
    mm1->mm2 boundary.
"""

import numpy as np

import concourse.mybir as mybir
import concourse.tile as tile
from concourse import bacc
from concourse import bass_utils

T, DIM, HID = 8192, 1024, 4096
N_CORES = 8
TOK_WAYS, HID_WAYS = 4, 2
T_S = T // TOK_WAYS        # 2048 tokens per core
HID_S = HID // HID_WAYS    # 2048 hidden units per core
P = 128
F32 = mybir.dt.float32
F16 = mybir.dt.float16

NSH = 4                    # moving chunks sharing one stationary load
T_CHUNK = T_S // NSH       # 512 tokens per moving chunk

KD = DIM // P              # 8 contraction chunks for mm1
JC = HID_S // P            # 16 j-chunks (also mm2 contraction chunks)
DC = DIM // P              # 8 output-dim chunks for mm2

def _ap_key(ap):
    return (str(ap.memsetref), str(ap.memref), ap.offset, str(ap.ap),
            str(ap.dtype))


def dedup_ldweights(nc):
    """Drop InstLdweights that reload the exact weights already resident in
    the PE array (legalization inserts one per matmul unconditionally)."""
    dropped = 0
    for f in nc.m.functions:
        for blk in f.blocks:
            insts = blk.instructions
            drop = []
            last_key = None
            for i, ins in enumerate(insts):
                nm = type(ins).__name__
                if nm == "InstLdweights":
                    si = ins.sync_info
                    clean = not si or (len(si.on_wait) == 0
                                       and len(si.on_update) == 0)
                    k = (_ap_key(ins.ins[0]), str(ins.perf_mode),
                         str(ins.is_transpose), str(ins.tile_position))
                    if k == last_key and clean:
                        drop.append(i)
                    else:
                        last_key = k
                elif nm == "InstMatmult":
                    if getattr(ins, "ldweights", True):
                        last_key = None
                elif getattr(ins, "engine", None) == mybir.EngineType.PE:
                    last_key = None
            for i in reversed(drop):
                del insts[i]
            dropped += len(drop)
    return dropped


def build_nc(reps: int = 1, full_loop: bool = False):
    nc = bacc.Bacc("TRN2", target_bir_lowering=False, debug=False)
    x2 = nc.dram_tensor("x2", [P, KD, T_S], F16, kind="ExternalInput")
    wfc2 = nc.dram_tensor("wfc2", [P, JC, KD, P], F16, kind="ExternalInput")
    wpj2 = nc.dram_tensor("wpj2", [P, JC, DIM], F16, kind="ExternalInput")
    outT = nc.dram_tensor("outT", [DIM, T_S], F16, kind="ExternalOutput")
    outT_r = outT.ap().rearrange("(o p) t -> p o t", p=P)

    with tile.TileContext(nc) as tc:
        with (
            tc.tile_pool(name="weights", bufs=1) as wpool,
            tc.tile_pool(name="xin", bufs=1) as xpool,
            tc.tile_pool(name="hact", bufs=1) as hpool,
            tc.tile_pool(name="tmp", bufs=3) as tpool,
            tc.tile_pool(name="outp", bufs=3) as opool,
            tc.tile_pool(name="ps_h", bufs=1, space="PSUM") as ps_h_pool,
            tc.tile_pool(name="ps_o", bufs=1, space="PSUM") as ps_o_pool,
        ):
            wfc_sb = wpool.tile([P, JC, KD, P], F16)
            wproj_sb = wpool.tile([P, JC, DIM], F16)
            x_sb = xpool.tile([P, KD, T_S], F16)

            # DMA issue order = need order for the interleaved j0/j1 start:
            # wfc[0], wfc[1] (the first stationary loads, needed at ~0.3us),
            # x k-major (k=0 split per t-chunk so the first matmul gates on
            # 128KB), then the remaining wfc j-slices, then wproj (needed
            # only when mm2 starts ~halfway in).
            def load_inputs():
                nc.sync.dma_start(wfc_sb[:, 0], wfc2.ap()[:, 0])
                nc.sync.dma_start(wfc_sb[:, 1], wfc2.ap()[:, 1])
                for t in range(NSH):
                    tsl = slice(t * T_CHUNK, (t + 1) * T_CHUNK)
                    nc.sync.dma_start(x_sb[:, 0, tsl], x2.ap()[:, 0, tsl])
                for k in range(1, KD):
                    nc.sync.dma_start(x_sb[:, k, :], x2.ap()[:, k, :])
                for j in range(2, JC):
                    nc.sync.dma_start(wfc_sb[:, j], wfc2.ap()[:, j])
                for j in range(JC):
                    nc.sync.dma_start(wproj_sb[:, j, :], wpj2.ap()[:, j, :])

            if not full_loop:
                load_inputs()

            def mm1_psum(j):
                pool_j = ps_h_pool if j % 2 == 0 else ps_o_pool
                pfx = "psh" if j % 2 == 0 else "pso"
                return [pool_j.tile([P, T_CHUNK], F32, tag=f"{pfx}{t}",
                                    name=f"{pfx}{t}") for t in range(NSH)]

            def mm1_k(j, k, pss):
                for t in range(NSH):
                    nc.tensor.matmul(
                        pss[t][:],
                        lhsT=wfc_sb[:, j, k, :],
                        rhs=x_sb[:, k, t * T_CHUNK:(t + 1) * T_CHUNK],
                        start=(k == 0), stop=(k == KD - 1),
                    )

            def mm1_act(j, pss, h_sb):
                for t in range(NSH):
                    relu_t = tpool.tile([P, T_CHUNK], F16, tag="relu")
                    nc.scalar.activation(
                        relu_t[:], pss[t][:],
                        mybir.ActivationFunctionType.Relu,
                    )
                    nc.vector.tensor_mul(
                        out=h_sb[:, j, t * T_CHUNK:(t + 1) * T_CHUNK],
                        in0=relu_t[:], in1=relu_t[:],
                    )

            def body(_iv=None):
                if full_loop:
                    load_inputs()
                h_sb = hpool.tile([P, JC, T_S], F16, tag="h")
                # j=0 and j=1 interleaved k-wise: the PE consumes x k-slices
                # at half the DMA delivery rate, so real matmuls start ~0.4us
                # in (first 128KB slice) and never starve on the x prologue.
                ps0, ps1 = mm1_psum(0), mm1_psum(1)
                for k in range(KD):
                    mm1_k(0, k, ps0)
                    mm1_k(1, k, ps1)
                mm1_act(0, ps0, h_sb)
                mm1_act(1, ps1, h_sb)
                for j in range(2, JC):
                    pss = mm1_psum(j)
                    for k in range(KD):
                        mm1_k(j, k, pss)
                    mm1_act(j, pss, h_sb)

                # j=JC-1 (odd) used ps_o; start mm2 on ps_h to avoid WAW.
                for dc in range(DC):
                    pool_d = ps_h_pool if dc % 2 == 0 else ps_o_pool
                    pfx = "psh" if dc % 2 == 0 else "pso"
                    pos = [pool_d.tile([P, T_CHUNK], F32, tag=f"{pfx}{t}",
                                       name=f"{pfx}{t}") for t in range(NSH)]
                    for j in range(JC):
                        for t in range(NSH):
                            nc.tensor.matmul(
                                pos[t][:],
                                lhsT=wproj_sb[:, j, dc * P:(dc + 1) * P],
                                rhs=h_sb[:, j, t * T_CHUNK:(t + 1) * T_CHUNK],
                                start=(j == 0), stop=(j == JC - 1),
                            )
                    for t in range(NSH):
                        o_sb = opool.tile([P, T_CHUNK], F16, tag="o")
                        # Alternate copy engines: halves the drain-chain depth
                        # per group (ScalarE is idle in the mm2 phase; Copy
                        # shares the Relu act-table so no table-switch cost).
                        if t % 2 == 0:
                            nc.vector.tensor_copy(out=o_sb[:], in_=pos[t][:])
                        else:
                            nc.scalar.activation(
                                o_sb[:], pos[t][:],
                                mybir.ActivationFunctionType.Copy,
                            )
                        nc.sync.dma_start(
                            outT_r[:, dc, t * T_CHUNK:(t + 1) * T_CHUNK],
                            o_sb[:],
                        )

            body()
            if reps > 1:
                with tc.For_i(0, reps - 1, 1) as iv:
                    body(iv)

    dedup_ldweights(nc)
    nc.compile()
    return nc


_NC_CACHE = {}


def _get_nc(reps: int = 1):
    if reps not in _NC_CACHE:
        _NC_CACHE[reps] = build_nc(reps)
    return _NC_CACHE[reps]


def make_in_maps(x, W_fc, W_proj):
    x16 = np.asarray(x, np.float32).astype(np.float16)
    wfc16 = np.asarray(W_fc, np.float32).astype(np.float16)
    wpj16 = np.asarray(W_proj, np.float32).astype(np.float16)
    in_maps = []
    for c in range(N_CORES):
        tok, hid = c // HID_WAYS, c % HID_WAYS
        t0 = tok * T_S
        h0 = hid * HID_S
        # x2[p,k,t] = x[t0+t, k*128+p]
        xs = x16[t0:t0 + T_S, :]
        x2 = np.ascontiguousarray(xs.T.reshape(KD, P, T_S).transpose(1, 0, 2))
        # wfc2[p,j,k,c] = W_fc[h0+j*128+c, k*128+p]
        wf = wfc16[h0:h0 + HID_S, :]
        wfc2 = np.ascontiguousarray(
            wf.reshape(JC, P, KD, P).transpose(3, 0, 2, 1))
        # wpj2[p,j,d] = W_proj[d, h0+j*128+p]
        wp = wpj16[:, h0:h0 + HID_S]
        wpj2 = np.ascontiguousarray(wp.T.reshape(JC, P, DIM).transpose(1, 0, 2))
        in_maps.append({"x2": x2, "wfc2": wfc2, "wpj2": wpj2})
    return in_maps


def assemble_out(results):
    out = np.empty((T, DIM), dtype=np.float32)
    for tok in range(TOK_WAYS):
        acc = results[tok * HID_WAYS]["outT"].astype(np.float32)
        for hid in range(1, HID_WAYS):
            acc += results[tok * HID_WAYS + hid]["outT"].astype(np.float32)
        out[tok * T_S:(tok + 1) * T_S] = acc.T
    return out


def kernel(x, W_fc, W_proj):
    assert x.shape == (T, DIM) and W_fc.shape == (HID, DIM) and W_proj.shape == (DIM, HID)
    nc = _get_nc(reps=1)
    in_maps = make_in_maps(x, W_fc, W_proj)
    res = bass_utils.run_bass_kernel_spmd(nc, in_maps, core_ids=list(range(N_CORES)))
    return assemble_out(res.results)


# revision 5
# speedup vs baseline: 1.0530x; 1.0530x over previous
"""Trainium2 Bass kernel for ExpertMLP: out = relu(x @ W_fc.T)^2 @ W_proj.T.

Sharding: 4-way tokens x 2-way hidden across 8 NeuronCores; host sums the
two fp16 hidden-half partials per token while unsharding.

Per-core kernel (T_S=2048 tokens, HID_S=2048 hidden, DIM=1024), fp16
operands, fp32 PSUM accumulation:
  mm1: h^T[j, t] = wfc-chunk.T @ x-chunks     (PSUM accum over d)
  act: relu^2 (ScalarE relu PSUM->SBUF fp16, VectorE square)
  mm2: out^T[d, t] = wproj-chunk.T @ h^T-chunks (PSUM accum over j)

Key optimizations vs naive:
  - Stationary sharing with REAL LDWEIGHTS dedup: bass legalization inserts
    an InstLdweights before every matmul; dedup_ldweights() deletes the
    redundant ones post-scheduling so each 128x128 weight chunk is loaded
    once per 4 streaming matmuls (hardware retains PE-array weights).
  - DRAM layouts pre-swizzled on host so every DMA is large and contiguous
    per partition; DMA issue order = need order (wfc[0:2] first, then x
    k-major with k=0 split per t-chunk) and mm1's first two j-groups are
    interleaved k-wise, so real matmuls start ~0.4us in and the PE never
    starves on the x prologue (no dummy warmup matmuls needed).
  - fp16 partial outputs (halves output DMA; host sums in fp32).
  - PSUM pool parity flipped between phases to avoid a WAW stall at the
    mm1->mm2 boundary.
"""

import numpy as np

import concourse.mybir as mybir
import concourse.tile as tile
from concourse import bacc
from concourse import bass_utils

T, DIM, HID = 8192, 1024, 4096
N_CORES = 8
TOK_WAYS, HID_WAYS = 4, 2
T_S = T // TOK_WAYS        # 2048 tokens per core
HID_S = HID // HID_WAYS    # 2048 hidden units per core
P = 128
F32 = mybir.dt.float32
F16 = mybir.dt.float16

NSH = 4                    # moving chunks sharing one stationary load
T_CHUNK = T_S // NSH       # 512 tokens per moving chunk

KD = DIM // P              # 8 contraction chunks for mm1
JC = HID_S // P            # 16 j-chunks (also mm2 contraction chunks)
DC = DIM // P              # 8 output-dim chunks for mm2

def _ap_key(ap):
    return (str(ap.memsetref), str(ap.memref), ap.offset, str(ap.ap),
            str(ap.dtype))


def dedup_ldweights(nc):
    """Drop InstLdweights that reload the exact weights already resident in
    the PE array (legalization inserts one per matmul unconditionally)."""
    dropped = 0
    for f in nc.m.functions:
        for blk in f.blocks:
            insts = blk.instructions
            drop = []
            last_key = None
            for i, ins in enumerate(insts):
                nm = type(ins).__name__
                if nm == "InstLdweights":
                    si = ins.sync_info
                    clean = not si or (len(si.on_wait) == 0
                                       and len(si.on_update) == 0)
                    k = (_ap_key(ins.ins[0]), str(ins.perf_mode),
                         str(ins.is_transpose), str(ins.tile_position))
                    if k == last_key and clean:
                        drop.append(i)
                    else:
                        last_key = k
                elif nm == "InstMatmult":
                    if getattr(ins, "ldweights", True):
                        last_key = None
                elif getattr(ins, "engine", None) == mybir.EngineType.PE:
                    last_key = None
            for i in reversed(drop):
                del insts[i]
            dropped += len(drop)
    return dropped


def build_nc(reps: int = 1, full_loop: bool = False):
    nc = bacc.Bacc("TRN2", target_bir_lowering=False, debug=False)
    x2 = nc.dram_tensor("x2", [P, KD, T_S], F16, kind="ExternalInput")
    wfc2 = nc.dram_tensor("wfc2", [P, JC, KD, P], F16, kind="ExternalInput")
    wpj2 = nc.dram_tensor("wpj2", [P, JC, DIM], F16, kind="ExternalInput")
    outT = nc.dram_tensor("outT", [DIM, T_S], F16, kind="ExternalOutput")
    outT_r = outT.ap().rearrange("(o p) t -> p o t", p=P)

    with tile.TileContext(nc) as tc:
        with (
            tc.tile_pool(name="weights", bufs=1) as wpool,
            tc.tile_pool(name="xin", bufs=1) as xpool,
            tc.tile_pool(name="hact", bufs=1) as hpool,
            tc.tile_pool(name="tmp", bufs=3) as tpool,
            tc.tile_pool(name="outp", bufs=3) as opool,
            tc.tile_pool(name="ps_h", bufs=1, space="PSUM") as ps_h_pool,
            tc.tile_pool(name="ps_o", bufs=1, space="PSUM") as ps_o_pool,
        ):
            wfc_sb = wpool.tile([P, JC, KD, P], F16)
            wproj_sb = wpool.tile([P, JC, DIM], F16)
            x_sb = xpool.tile([P, KD, T_S], F16)

            # DMA issue order = need order for the interleaved j0/j1 start:
            # wfc[0], wfc[1] (the first stationary loads, needed at ~0.3us),
            # x k-major (k=0 split per t-chunk so the first matmul gates on
            # 128KB), then the remaining wfc j-slices, then wproj (needed
            # only when mm2 starts ~halfway in).
            def load_inputs():
                nc.sync.dma_start(wfc_sb[:, 0], wfc2.ap()[:, 0])
                nc.sync.dma_start(wfc_sb[:, 1], wfc2.ap()[:, 1])
                for t in range(NSH):
                    tsl = slice(t * T_CHUNK, (t + 1) * T_CHUNK)
                    nc.sync.dma_start(x_sb[:, 0, tsl], x2.ap()[:, 0, tsl])
                for k in range(1, KD):
                    nc.sync.dma_start(x_sb[:, k, :], x2.ap()[:, k, :])
                for j in range(2, JC):
                    nc.sync.dma_start(wfc_sb[:, j], wfc2.ap()[:, j])
                for j in range(JC):
                    nc.sync.dma_start(wproj_sb[:, j, :], wpj2.ap()[:, j, :])

            if not full_loop:
                load_inputs()

            def mm1_psum(j):
                pool_j = ps_h_pool if j % 2 == 0 else ps_o_pool
                pfx = "psh" if j % 2 == 0 else "pso"
                return [pool_j.tile([P, T_CHUNK], F32, tag=f"{pfx}{t}",
                                    name=f"{pfx}{t}") for t in range(NSH)]

            def mm1_k(j, k, pss):
                for t in range(NSH):
                    nc.tensor.matmul(
                        pss[t][:],
                        lhsT=wfc_sb[:, j, k, :],
                        rhs=x_sb[:, k, t * T_CHUNK:(t + 1) * T_CHUNK],
                        start=(k == 0), stop=(k == KD - 1),
                    )

            def mm1_act(j, pss, h_sb):
                for t in range(NSH):
                    relu_t = tpool.tile([P, T_CHUNK], F16, tag="relu")
                    nc.scalar.activation(
                        relu_t[:], pss[t][:],
                        mybir.ActivationFunctionType.Relu,
                    )
                    nc.vector.tensor_mul(
                        out=h_sb[:, j, t * T_CHUNK:(t + 1) * T_CHUNK],
                        in0=relu_t[:], in1=relu_t[:],
                    )

            def body(_iv=None):
                if full_loop:
                    load_inputs()
                h_sb = hpool.tile([P, JC, T_S], F16, tag="h")
                # j=0 and j=1 interleaved k-wise: the PE consumes x k-slices
                # at half the DMA delivery rate, so real matmuls start ~0.4us
                # in (first 128KB slice) and never starve on the x prologue.
                ps0, ps1 = mm1_psum(0), mm1_psum(1)
                for k in range(KD):
                    mm1_k(0, k, ps0)
                    mm1_k(1, k, ps1)
                mm1_act(0, ps0, h_sb)
                mm1_act(1, ps1, h_sb)
                for j in range(2, JC):
                    pss = mm1_psum(j)
                    for k in range(KD):
                        mm1_k(j, k, pss)
                    mm1_act(j, pss, h_sb)

                # j=JC-1 (odd) used ps_o; start mm2 on ps_h to avoid WAW.
                for dc in range(DC):
                    pool_d = ps_h_pool if dc % 2 == 0 else ps_o_pool
                    pfx = "psh" if dc % 2 == 0 else "pso"
                    pos = [pool_d.tile([P, T_CHUNK], F32, tag=f"{pfx}{t}",
                                       name=f"{pfx}{t}") for t in range(NSH)]
                    for j in range(JC):
                        for t in range(NSH):
                            nc.tensor.matmul(
                                pos[t][:],
                                lhsT=wproj_sb[:, j, dc * P:(dc + 1) * P],
                                rhs=h_sb[:, j, t * T_CHUNK:(t + 1) * T_CHUNK],
                                start=(j == 0), stop=(j == JC - 1),
                            )
                    for t in range(NSH):
                        o_sb = opool.tile([P, T_CHUNK], F16, tag="o")
                        nc.vector.tensor_copy(out=o_sb[:], in_=pos[t][:])
                        nc.sync.dma_start(
                            outT_r[:, dc, t * T_CHUNK:(t + 1) * T_CHUNK],
                            o_sb[:],
                        )

            body()
            if reps > 1:
                with tc.For_i(0, reps - 1, 1) as iv:
                    body(iv)

    dedup_ldweights(nc)
    nc.compile()
    return nc


_NC_CACHE = {}


def _get_nc(reps: int = 1):
    if reps not in _NC_CACHE:
        _NC_CACHE[reps] = build_nc(reps)
    return _NC_CACHE[reps]


def make_in_maps(x, W_fc, W_proj):
    x16 = np.asarray(x, np.float32).astype(np.float16)
    wfc16 = np.asarray(W_fc, np.float32).astype(np.float16)
    wpj16 = np.asarray(W_proj, np.float32).astype(np.float16)
    in_maps = []
    for c in range(N_CORES):
        tok, hid = c // HID_WAYS, c % HID_WAYS
        t0 = tok * T_S
        h0 = hid * HID_S
        # x2[p,k,t] = x[t0+t, k*128+p]
        xs = x16[t0:t0 + T_S, :]
        x2 = np.ascontiguousarray(xs.T.reshape(KD, P, T_S).transpose(1, 0, 2))
        # wfc2[p,j,k,c] = W_fc[h0+j*128+c, k*128+p]
        wf = wfc16[h0:h0 + HID_S, :]
        wfc2 = np.ascontiguousarray(
            wf.reshape(JC, P, KD, P).transpose(3, 0, 2, 1))
        # wpj2[p,j,d] = W_proj[d, h0+j*128+p]
        wp = wpj16[:, h0:h0 + HID_S]
        wpj2 = np.ascontiguousarray(wp.T.reshape(JC, P, DIM).transpose(1, 0, 2))
        in_maps.append({"x2": x2, "wfc2": wfc2, "wpj2": wpj2})
    return in_maps


def assemble_out(results):
    out = np.empty((T, DIM), dtype=np.float32)
    for tok in range(TOK_WAYS):
        acc = results[tok * HID_WAYS]["outT"].astype(np.float32)
        for hid in range(1, HID_WAYS):
            acc += results[tok * HID_WAYS + hid]["outT"].astype(np.float32)
        out[tok * T_S:(tok + 1) * T_S] = acc.T
    return out


def kernel(x, W_fc, W_proj):
    assert x.shape == (T, DIM) and W_fc.shape == (HID, DIM) and W_proj.shape == (DIM, HID)
    nc = _get_nc(reps=1)
    in_maps = make_in_maps(x, W_fc, W_proj)
    res = bass_utils.run_bass_kernel_spmd(nc, in_maps, core_ids=list(range(N_CORES)))
    return assemble_out(res.results)
